# revision 28
# baseline (speedup 1.0000x reference)
"""MoE-routed 3x3 conv (MixedLayerWithArc) on 8 TRN2 NeuronCores.

Reference semantics: out[i] = conv3x3(x[i], W[sample_arc[i]], b[sample_arc[i]])
(the dense all-branch + one-hot-mask reference computes exactly this).

Strategy:
  * Routing resolved on the HOST (sample_arc is host data): gather the
    selected branch's weights/bias per sample -> 1 conv per sample instead
    of 4 (4x less compute than the reference).
  * Data-parallel over batch: 8 samples per core x 8 cores.
  * Host groups same-branch samples into pairs: each core gets 3 pairs + 2
    singles -> 5 weight slots instead of 8 (weight DMA 18.9 -> 11.8 MB/core).
    Among 64 samples in 4 branches there are always >= 30 same-branch pairs,
    so 24 pairs for 8 cores always exist.
  * Conv as 9 shifted matmuls accumulated in PSUM, contracting over C_in
    (256 = 2 partition tiles of 128). dtype float32r: 4x PE throughput vs
    float32 at ~1e-4 relative error.
  * Host pre-pads x to 34x34 so DMAs are contiguous and each tap is a
    strided SBUF view; bias folds into the PSUM->SBUF eviction.
  * PE warmup matmuls run during the initial DMA fill (HAM clock ramp).

Per-core inputs:
  xp  [8, 2, 128, 34, 34] f32   padded input   (sample, ci_tile, ci, h, w)
  wt  [5, 2, 128, 9, 2, 128] f32 weights       (slot, ci_tile, ci, tap, co_tile, co)
  bs  [128, 16] f32              bias          (co, sample*2 + co_tile)
  out [8, 2, 128, 1024] f32                    (sample, co_tile, co, h*w)
"""
import numpy as np

B, C, H, W_ = 64, 256, 32, 32
NCORES = 8
SPC = B // NCORES          # samples per core
HP, WP = H + 2, W_ + 2     # padded spatial
P = 128                    # partition tile
CT = C // P                # channel tiles (2)
NHALF = H * W_ // 2        # 512 = one PSUM bank of fp32
NSLOT = 5                  # weight slots per core (3 pairs + 2 singles)
SLOT_OF = [0, 0, 1, 1, 2, 2, 3, 4]   # sample -> weight slot (static)
WARMUP = 16

TRACE = False
TRACE_DIR = None
LAST_RESULTS = None

_prog_cache = {}


def _build_program():
    import concourse.tile as tile
    from concourse import bacc, mybir

    nc = bacc.Bacc("TRN2", target_bir_lowering=False, debug=False,
                   num_devices=NCORES)
    f32 = mybir.dt.float32
    f32r = mybir.dt.float32r

    xp_d = nc.dram_tensor("xp", [SPC, CT, P, HP, WP], f32r,
                          kind="ExternalInput").ap()
    wt_d = nc.dram_tensor("wt", [NSLOT, CT, P, 9, CT, P], f32r,
                          kind="ExternalInput").ap()
    bs_d = nc.dram_tensor("bs", [P, SPC * CT], f32,
                          kind="ExternalInput").ap()
    out_d = nc.dram_tensor("out", [SPC, CT, P, H * W_], f32,
                           kind="ExternalOutput").ap()

    with tile.TileContext(nc) as tc:
        with tc.tile_pool(name="xpool", bufs=3) as xpool, \
             tc.tile_pool(name="wpool", bufs=4) as wpool, \
             tc.tile_pool(name="bpool", bufs=1) as bpool, \
             tc.tile_pool(name="opool", bufs=8) as opool, \
             tc.tile_pool(name="psum", bufs=8, space="PSUM") as psum_pool:

            # PE warmup: dummy fp32 matmuls on a memset tile keep the PE
            # busy during the initial DMA fill so the HAM clock gate opens
            # (1.2 -> 2.4 GHz) before the first real matmul.
            scratch = bpool.tile([P, P], f32, name="scratch")
            nc.gpsimd.memset(scratch[:], 0.0)
            ps_warm = psum_pool.tile([P, NHALF], f32, name="ps_warm", tag="ps")
            for _ in range(WARMUP):
                nc.tensor.matmul(ps_warm[:, :P], scratch[:], scratch[:],
                                 start=True, stop=True, skip_group_check=True)

            bt = bpool.tile([P, SPC * CT], f32)


            def tap_aps(xts, ci_t, tap, ch, pstile):
                # Output row 0 (chunk 0) reads only the zero pad row for
                # dy=0 taps, row 31 (chunk 1) only for dy=2: shrink those
                # matmuls to 15 rows (N=480). start=True clears has_written
                # for the whole bank, and the full-width dy=1 taps overwrite
                # the untouched columns, so partial-range accumulation is
                # sound.
                dy, dx = divmod(tap, 3)
                r0, r1 = 16 * ch, 16 * ch + 16
                c0, c1 = 0, NHALF
                if ch == 0 and dy == 0:
                    r0, c0 = r0 + 1, 32
                elif ch == 1 and dy == 2:
                    r1, c1 = r1 - 1, NHALF - 32
                rhs = xts[ci_t][:, dy + r0: dy + r1, dx: dx + W_]
                return rhs, pstile[:, c0:c1]

            wslots = {}

            def load_wslot(slot, split):
                tiles = []
                for ci_t in range(CT):
                    wtile = wpool.tile([P, 9, CT, P], f32r,
                                       name=f"wt{slot}_{ci_t}", tag="wt")
                    if split:
                        # three pieces: the first matmuls only need low taps
                        for g in range(3):
                            nc.scalar.dma_start(
                                wtile[:, 3 * g: 3 * g + 3],
                                wt_d[slot, ci_t][:, 3 * g: 3 * g + 3])
                    else:
                        nc.scalar.dma_start(wtile[:], wt_d[slot, ci_t])
                    tiles.append(wtile)
                wslots[slot] = tiles
                return tiles

            for s in range(SPC):
                xts = []
                for ci_t in range(CT):
                    xt = xpool.tile([P, HP, WP], f32r, name=f"xt{s}_{ci_t}",
                                    tag="xt")
                    if s == 0:
                        # row-split so the chunk-0 matmuls start sooner
                        nc.sync.dma_start(xt[:, :18], xp_d[s, ci_t][:, :18])
                        nc.sync.dma_start(xt[:, 18:], xp_d[s, ci_t][:, 18:])
                    else:
                        nc.sync.dma_start(xt[:], xp_d[s, ci_t])
                    xts.append(xt)
                slot = SLOT_OF[s]
                wts = wslots.get(slot) or load_wslot(slot, split=(s == 0))
                if s == 0:
                    nc.scalar.dma_start(bt[:], bs_d[:])

                ps = [[psum_pool.tile([P, NHALF], f32,
                                      name=f"ps{s}_{co_t}_{ch}", tag="ps")
                       for ch in range(2)] for co_t in range(CT)]

                last = s == SPC - 1
                if s == 0:
                    # (ch,ci) staged: first 18 matmuls only need x rows 0-17
                    # + ci0 weights (earliest start); ci1 not needed until
                    # half the sample is done (long prefetch window)
                    for ch, ci_t in ((0, 0), (1, 0), (0, 1), (1, 1)):
                        for tap in range(9):
                            for co_t in range(CT):
                                rhs, out_ap = tap_aps(xts, ci_t, tap, ch,
                                                      ps[co_t][ch])
                                nc.tensor.matmul(
                                    rhs=rhs, out=out_ap,
                                    lhsT=wts[ci_t][:, tap, co_t, :],
                                    start=(ci_t == 0 and tap == 0),
                                    stop=(ci_t == CT - 1 and tap == 8))
                    groups = [(co_t, ch) for co_t in range(CT)
                              for ch in range(2)]
                    mm_done = True
                elif not last:
                    # ci-outer: only ci_t=0 tiles gate the first 36 matmuls
                    for ci_t in range(CT):
                        for tap in range(9):
                            for co_t in range(CT):
                                lhsT = wts[ci_t][:, tap, co_t, :]
                                for ch in range(2):
                                    rhs, out_ap = tap_aps(xts, ci_t, tap, ch,
                                                          ps[co_t][ch])
                                    nc.tensor.matmul(
                                        rhs=rhs, out=out_ap, lhsT=lhsT,
                                        start=(ci_t == 0 and tap == 0),
                                        stop=(ci_t == CT - 1 and tap == 8))
                    groups = [(co_t, ch) for co_t in range(CT)
                              for ch in range(2)]
                    mm_done = True
                else:
                    groups = [(co_t, ch) for co_t in range(CT)
                              for ch in range(2)]
                    mm_done = False

                for co_t, ch in groups:
                    if not mm_done:
                        # last sample: emit each psum group's matmuls just
                        # before its drain so only one group lands in the tail
                        for ci_t in range(CT):
                            for tap in range(9):
                                rhs, out_ap = tap_aps(xts, ci_t, tap, ch,
                                                      ps[co_t][ch])
                                nc.tensor.matmul(
                                    rhs=rhs, out=out_ap,
                                    lhsT=wts[ci_t][:, tap, co_t, :],
                                    start=(ci_t == 0 and tap == 0),
                                    stop=(ci_t == CT - 1 and tap == 8))
                    ot = opool.tile([P, NHALF], f32,
                                    name=f"ot{s}_{co_t}_{ch}", tag="ot")
                    nc.vector.tensor_scalar_add(
                        ot[:], ps[co_t][ch][:],
                        bt[:, CT * s + co_t: CT * s + co_t + 1])
                    nc.sync.dma_start(
                        out_d[s, co_t][:, NHALF * ch: NHALF * (ch + 1)],
                        ot[:])
    nc.compile()
    return nc


def _plan_routing(arc):
    """Group the 64 samples into 24 same-branch pairs + 16 singles and lay
    them out per core as [p0,p0,p1,p1,p2,p2,s0,s1]. Returns (perm, wslot_src)
    where perm[core*8+pos] = original sample index and wslot_src[core*5+k] =
    original sample whose branch fills weight slot k of that core."""
    groups = [list(np.nonzero(arc == b)[0]) for b in range(4)]
    pairs, singles = [], []
    for g in groups:
        n2 = (len(g) // 2) * 2
        pairs.extend((g[i], g[i + 1]) for i in range(0, n2, 2))
        singles.extend(g[n2:])
    # move surplus pairs beyond 24 back to singles (keep exactly 24 pairs)
    while len(pairs) > 3 * NCORES:
        a, bb = pairs.pop()
        singles.extend([a, bb])
    assert len(pairs) == 3 * NCORES and len(singles) == 2 * NCORES
    perm = np.empty(B, np.int64)
    wsrc = np.empty(NCORES * NSLOT, np.int64)
    for c in range(NCORES):
        ps_ = pairs[3 * c: 3 * c + 3]
        sg = singles[2 * c: 2 * c + 2]
        samp = [ps_[0][0], ps_[0][1], ps_[1][0], ps_[1][1],
                ps_[2][0], ps_[2][1], sg[0], sg[1]]
        perm[8 * c: 8 * c + 8] = samp
        wsrc[NSLOT * c: NSLOT * (c + 1)] = [ps_[0][0], ps_[1][0], ps_[2][0],
                                            sg[0], sg[1]]
    return perm, wsrc


def kernel(x, sample_arc, W, b):
    global LAST_RESULTS
    from concourse.bass_utils import run_bass_kernel_spmd

    x = np.asarray(x, dtype=np.float32)
    arc = np.asarray(sample_arc)
    W = np.asarray(W, dtype=np.float32)
    b = np.asarray(b, dtype=np.float32)

    nc = _prog_cache.get("nc")
    if nc is None:
        nc = _prog_cache["nc"] = _build_program()

    perm, wsrc = _plan_routing(arc)

    # packed x (padded), in permuted order
    xp = np.zeros((B, CT, P, HP, WP), np.float32)
    xp[:, :, :, 1:1 + H, 1:1 + W_] = x[perm].reshape(B, CT, P, H, W_)

    # per-slot weights: [ci, ky, kx, co] layout
    wsel = W[arc[wsrc]]                      # [40, co, ci, 3, 3]
    wt = np.ascontiguousarray(wsel.transpose(0, 2, 3, 4, 1))
    wt = wt.reshape(NCORES, NSLOT, CT, P, 9, CT, P)

    bsel = b[arc[perm]].reshape(NCORES, SPC, CT, P)

    in_maps = []
    for c in range(NCORES):
        in_maps.append({
            "xp": xp[c * SPC:(c + 1) * SPC],
            "wt": wt[c],
            "bs": np.ascontiguousarray(
                bsel[c].transpose(2, 0, 1).reshape(P, SPC * CT)),
        })

    res = run_bass_kernel_spmd(nc, in_maps, core_ids=list(range(NCORES)),
                               trace=TRACE, tmpdir=TRACE_DIR)
    LAST_RESULTS = res

    out_perm = np.concatenate(
        [res.results[c]["out"].reshape(SPC, C, H, W_) for c in range(NCORES)],
        axis=0)
    out = np.empty_like(out_perm)
    out[perm] = out_perm
    return out


# revision 29
# speedup vs baseline: 1.0177x; 1.0177x over previous
"""MoE-routed 3x3 conv (MixedLayerWithArc) on 8 TRN2 NeuronCores.

Reference semantics: out[i] = conv3x3(x[i], W[sample_arc[i]], b[sample_arc[i]])
(the dense all-branch + one-hot-mask reference computes exactly this).

Strategy:
  * Routing resolved on the HOST (sample_arc is host data): gather the
    selected branch's weights/bias per sample -> 1 conv per sample instead
    of 4 (4x less compute than the reference).
  * Data-parallel over batch: 8 samples per core x 8 cores.
  * Host groups same-branch samples into pairs: each core gets 3 pairs + 2
    singles -> 5 weight slots instead of 8 (weight DMA 18.9 -> 11.8 MB/core).
    Among 64 samples in 4 branches there are always >= 30 same-branch pairs,
    so 24 pairs for 8 cores always exist.
  * Conv as 9 shifted matmuls accumulated in PSUM, contracting over C_in
    (256 = 2 partition tiles of 128). dtype float32r: 4x PE throughput vs
    float32 at ~1e-4 relative error.
  * Host pre-pads x to 34x34 so DMAs are contiguous and each tap is a
    strided SBUF view; bias folds into the PSUM->SBUF eviction.
  * PE warmup matmuls run during the initial DMA fill (HAM clock ramp).

Per-core inputs:
  xp  [8, 2, 128, 34, 34] f32   padded input   (sample, ci_tile, ci, h, w)
  wt  [5, 2, 128, 9, 2, 128] f32 weights       (slot, ci_tile, ci, tap, co_tile, co)
  bs  [128, 16] f32              bias          (co, sample*2 + co_tile)
  out [8, 2, 128, 1024] f32                    (sample, co_tile, co, h*w)
"""
import numpy as np

B, C, H, W_ = 64, 256, 32, 32
NCORES = 8
SPC = B // NCORES          # samples per core
HP, WP = H + 2, W_ + 2     # padded spatial
P = 128                    # partition tile
CT = C // P                # channel tiles (2)
NHALF = H * W_ // 2        # 512 = one PSUM bank of fp32
NSLOT = 5                  # weight slots per core (3 pairs + 2 singles)
SLOT_OF = [0, 0, 1, 1, 2, 2, 3, 4]   # sample -> weight slot (static)
WARMUP = 16

TRACE = False
TRACE_DIR = None
LAST_RESULTS = None

_prog_cache = {}


def _build_program():
    import concourse.tile as tile
    from concourse import bacc, mybir

    nc = bacc.Bacc("TRN2", target_bir_lowering=False, debug=False,
                   num_devices=NCORES)
    f32 = mybir.dt.float32
    f32r = mybir.dt.float32r

    xp_d = nc.dram_tensor("xp", [SPC, CT, P, HP, WP], f32r,
                          kind="ExternalInput").ap()
    wt_d = nc.dram_tensor("wt", [NSLOT, CT, P, 9, CT, P], f32r,
                          kind="ExternalInput").ap()
    bs_d = nc.dram_tensor("bs", [P, SPC * CT], f32,
                          kind="ExternalInput").ap()
    out_d = nc.dram_tensor("out", [SPC, CT, P, H * W_], f32,
                           kind="ExternalOutput").ap()

    with tile.TileContext(nc) as tc:
        with tc.tile_pool(name="xpool", bufs=3) as xpool, \
             tc.tile_pool(name="xfpool", bufs=3) as xfpool, \
             tc.tile_pool(name="wpool", bufs=4) as wpool, \
             tc.tile_pool(name="bpool", bufs=1) as bpool, \
             tc.tile_pool(name="opool", bufs=8) as opool, \
             tc.tile_pool(name="psum", bufs=8, space="PSUM") as psum_pool:

            # PE warmup: dummy fp32 matmuls on a memset tile keep the PE
            # busy during the initial DMA fill so the HAM clock gate opens
            # (1.2 -> 2.4 GHz) before the first real matmul.
            scratch = bpool.tile([P, P], f32, name="scratch")
            nc.gpsimd.memset(scratch[:], 0.0)
            ps_warm = psum_pool.tile([P, NHALF], f32, name="ps_warm", tag="ps")
            for _ in range(WARMUP):
                nc.tensor.matmul(ps_warm[:, :P], scratch[:], scratch[:],
                                 start=True, stop=True, skip_group_check=True)

            bt = bpool.tile([P, SPC * CT], f32)


            def tap_aps(xts, ci_t, tap, ch, pstile, xfs=None):
                # Output row 0 (chunk 0) reads only the zero pad row for
                # dy=0 taps, row 31 (chunk 1) only for dy=2: shrink those
                # matmuls to 15 rows (N=480). start=True clears has_written
                # for the whole bank, and the full-width dy=1 taps overwrite
                # the untouched columns, so partial-range accumulation is
                # sound.
                dy, dx = divmod(tap, 3)
                r0, r1 = 16 * ch, 16 * ch + 16
                c0, c1 = 0, NHALF
                if ch == 0 and dy == 0:
                    r0, c0 = r0 + 1, 32
                elif ch == 1 and dy == 2:
                    r1, c1 = r1 - 1, NHALF - 32
                if xfs is not None and dx == 1:
                    # center-column taps read the unpadded flat copy: a 1-D
                    # rhs AP is ~9ns/MM cheaper than the strided window
                    f0 = (dy - 1 + r0) * W_
                    rhs = xfs[ci_t][:, f0: f0 + (r1 - r0) * W_]
                else:
                    rhs = xts[ci_t][:, dy + r0: dy + r1, dx: dx + W_]
                return rhs, pstile[:, c0:c1]

            wslots = {}

            def load_wslot(slot, split):
                tiles = []
                for ci_t in range(CT):
                    wtile = wpool.tile([P, 9, CT, P], f32r,
                                       name=f"wt{slot}_{ci_t}", tag="wt")
                    if split:
                        # three pieces: the first matmuls only need low taps
                        for g in range(3):
                            nc.scalar.dma_start(
                                wtile[:, 3 * g: 3 * g + 3],
                                wt_d[slot, ci_t][:, 3 * g: 3 * g + 3])
                    else:
                        nc.scalar.dma_start(wtile[:], wt_d[slot, ci_t])
                    tiles.append(wtile)
                wslots[slot] = tiles
                return tiles

            for s in range(SPC):
                xts = []
                for ci_t in range(CT):
                    xt = xpool.tile([P, HP, WP], f32r, name=f"xt{s}_{ci_t}",
                                    tag="xt")
                    if s == 0:
                        # row-split so the chunk-0 matmuls start sooner
                        nc.sync.dma_start(xt[:, :18], xp_d[s, ci_t][:, :18])
                        nc.sync.dma_start(xt[:, 18:], xp_d[s, ci_t][:, 18:])
                    else:
                        nc.sync.dma_start(xt[:], xp_d[s, ci_t])
                    xts.append(xt)
                xfs = None
                if s > 0:
                    xfs = []
                    for ci_t in range(CT):
                        xf = xfpool.tile([P, H * W_], f32r,
                                         name=f"xf{s}_{ci_t}", tag="xf")
                        nc.vector.tensor_copy(
                            xf[:], xts[ci_t][:, 1:1 + H, 1:1 + W_])
                        xfs.append(xf)
                slot = SLOT_OF[s]
                wts = wslots.get(slot) or load_wslot(slot, split=(s == 0))
                if s == 0:
                    nc.scalar.dma_start(bt[:], bs_d[:])

                ps = [[psum_pool.tile([P, NHALF], f32,
                                      name=f"ps{s}_{co_t}_{ch}", tag="ps")
                       for ch in range(2)] for co_t in range(CT)]

                last = s == SPC - 1
                if s == 0:
                    # (ch,ci) staged: first 18 matmuls only need x rows 0-17
                    # + ci0 weights (earliest start); ci1 not needed until
                    # half the sample is done (long prefetch window)
                    for ch, ci_t in ((0, 0), (1, 0), (0, 1), (1, 1)):
                        for tap in range(9):
                            for co_t in range(CT):
                                rhs, out_ap = tap_aps(xts, ci_t, tap, ch,
                                                      ps[co_t][ch])
                                nc.tensor.matmul(
                                    rhs=rhs, out=out_ap,
                                    lhsT=wts[ci_t][:, tap, co_t, :],
                                    start=(ci_t == 0 and tap == 0),
                                    stop=(ci_t == CT - 1 and tap == 8))
                    groups = [(co_t, ch) for co_t in range(CT)
                              for ch in range(2)]
                    mm_done = True
                elif not last:
                    # ci-outer: only ci_t=0 tiles gate the first 36 matmuls
                    for ci_t in range(CT):
                        for tap in range(9):
                            for co_t in range(CT):
                                lhsT = wts[ci_t][:, tap, co_t, :]
                                for ch in range(2):
                                    rhs, out_ap = tap_aps(xts, ci_t, tap, ch,
                                                          ps[co_t][ch], xfs)
                                    nc.tensor.matmul(
                                        rhs=rhs, out=out_ap, lhsT=lhsT,
                                        start=(ci_t == 0 and tap == 0),
                                        stop=(ci_t == CT - 1 and tap == 8))
                    groups = [(co_t, ch) for co_t in range(CT)
                              for ch in range(2)]
                    mm_done = True
                else:
                    groups = [(co_t, ch) for co_t in range(CT)
                              for ch in range(2)]
                    mm_done = False

                for co_t, ch in groups:
                    if not mm_done:
                        # last sample: emit each psum group's matmuls just
                        # before its drain so only one group lands in the tail
                        for ci_t in range(CT):
                            for tap in range(9):
                                rhs, out_ap = tap_aps(xts, ci_t, tap, ch,
                                                      ps[co_t][ch], xfs)
                                nc.tensor.matmul(
                                    rhs=rhs, out=out_ap,
                                    lhsT=wts[ci_t][:, tap, co_t, :],
                                    start=(ci_t == 0 and tap == 0),
                                    stop=(ci_t == CT - 1 and tap == 8))
                    ot = opool.tile([P, NHALF], f32,
                                    name=f"ot{s}_{co_t}_{ch}", tag="ot")
                    nc.vector.tensor_scalar_add(
                        ot[:], ps[co_t][ch][:],
                        bt[:, CT * s + co_t: CT * s + co_t + 1])
                    nc.sync.dma_start(
                        out_d[s, co_t][:, NHALF * ch: NHALF * (ch + 1)],
                        ot[:])
    nc.compile()
    return nc


def _plan_routing(arc):
    """Group the 64 samples into 24 same-branch pairs + 16 singles and lay
    them out per core as [p0,p0,p1,p1,p2,p2,s0,s1]. Returns (perm, wslot_src)
    where perm[core*8+pos] = original sample index and wslot_src[core*5+k] =
    original sample whose branch fills weight slot k of that core."""
    groups = [list(np.nonzero(arc == b)[0]) for b in range(4)]
    pairs, singles = [], []
    for g in groups:
        n2 = (len(g) // 2) * 2
        pairs.extend((g[i], g[i + 1]) for i in range(0, n2, 2))
        singles.extend(g[n2:])
    # move surplus pairs beyond 24 back to singles (keep exactly 24 pairs)
    while len(pairs) > 3 * NCORES:
        a, bb = pairs.pop()
        singles.extend([a, bb])
    assert len(pairs) == 3 * NCORES and len(singles) == 2 * NCORES
    perm = np.empty(B, np.int64)
    wsrc = np.empty(NCORES * NSLOT, np.int64)
    for c in range(NCORES):
        ps_ = pairs[3 * c: 3 * c + 3]
        sg = singles[2 * c: 2 * c + 2]
        samp = [ps_[0][0], ps_[0][1], ps_[1][0], ps_[1][1],
                ps_[2][0], ps_[2][1], sg[0], sg[1]]
        perm[8 * c: 8 * c + 8] = samp
        wsrc[NSLOT * c: NSLOT * (c + 1)] = [ps_[0][0], ps_[1][0], ps_[2][0],
                                            sg[0], sg[1]]
    return perm, wsrc


def kernel(x, sample_arc, W, b):
    global LAST_RESULTS
    from concourse.bass_utils import run_bass_kernel_spmd

    x = np.asarray(x, dtype=np.float32)
    arc = np.asarray(sample_arc)
    W = np.asarray(W, dtype=np.float32)
    b = np.asarray(b, dtype=np.float32)

    nc = _prog_cache.get("nc")
    if nc is None:
        nc = _prog_cache["nc"] = _build_program()

    perm, wsrc = _plan_routing(arc)

    # packed x (padded), in permuted order
    xp = np.zeros((B, CT, P, HP, WP), np.float32)
    xp[:, :, :, 1:1 + H, 1:1 + W_] = x[perm].reshape(B, CT, P, H, W_)

    # per-slot weights: [ci, ky, kx, co] layout
    wsel = W[arc[wsrc]]                      # [40, co, ci, 3, 3]
    wt = np.ascontiguousarray(wsel.transpose(0, 2, 3, 4, 1))
    wt = wt.reshape(NCORES, NSLOT, CT, P, 9, CT, P)

    bsel = b[arc[perm]].reshape(NCORES, SPC, CT, P)

    in_maps = []
    for c in range(NCORES):
        in_maps.append({
            "xp": xp[c * SPC:(c + 1) * SPC],
            "wt": wt[c],
            "bs": np.ascontiguousarray(
                bsel[c].transpose(2, 0, 1).reshape(P, SPC * CT)),
        })

    res = run_bass_kernel_spmd(nc, in_maps, core_ids=list(range(NCORES)),
                               trace=TRACE, tmpdir=TRACE_DIR)
    LAST_RESULTS = res

    out_perm = np.concatenate(
        [res.results[c]["out"].reshape(SPC, C, H, W_) for c in range(NCORES)],
        axis=0)
    out = np.empty_like(out_perm)
    out[perm] = out_perm
    return out


# revision 30
# speedup vs baseline: 1.0452x; 1.0270x over previous
"""MoE-routed 3x3 conv (MixedLayerWithArc) on 8 TRN2 NeuronCores.

Reference semantics: out[i] = conv3x3(x[i], W[sample_arc[i]], b[sample_arc[i]])
(the dense all-branch + one-hot-mask reference computes exactly this).

Strategy:
  * Routing resolved on the HOST (sample_arc is host data): gather the
    selected branch's weights/bias per sample -> 1 conv per sample instead
    of 4 (4x less compute than the reference).
  * Data-parallel over batch: 8 samples per core x 8 cores.
  * Host groups same-branch samples into pairs: each core gets 3 pairs + 2
    singles -> 5 weight slots instead of 8 (weight DMA 18.9 -> 11.8 MB/core).
    Among 64 samples in 4 branches there are always >= 30 same-branch pairs,
    so 24 pairs for 8 cores always exist.
  * Conv as 9 shifted matmuls accumulated in PSUM, contracting over C_in
    (256 = 2 partition tiles of 128). dtype float32r: 4x PE throughput vs
    float32 at ~1e-4 relative error.
  * Host pre-pads x to 34x34 so DMAs are contiguous and each tap is a
    strided SBUF view; bias folds into the PSUM->SBUF eviction.
  * PE warmup matmuls run during the initial DMA fill (HAM clock ramp).

Per-core inputs:
  xp  [8, 2, 128, 34, 34] f32   padded input   (sample, ci_tile, ci, h, w)
  wt  [5, 2, 128, 9, 2, 128] f32 weights       (slot, ci_tile, ci, tap, co_tile, co)
  bs  [128, 16] f32              bias          (co, sample*2 + co_tile)
  out [8, 2, 128, 1024] f32                    (sample, co_tile, co, h*w)
"""
import numpy as np

B, C, H, W_ = 64, 256, 32, 32
NCORES = 8
SPC = B // NCORES          # samples per core
HP, WP = H + 2, W_ + 2     # padded spatial
P = 128                    # partition tile
CT = C // P                # channel tiles (2)
NHALF = H * W_ // 2        # 512 = one PSUM bank of fp32
NSLOT = 5                  # weight slots per core (3 pairs + 2 singles)
SLOT_OF = [0, 0, 1, 1, 2, 2, 3, 4]   # sample -> weight slot (static)
WARMUP = 16

TRACE = False
TRACE_DIR = None
LAST_RESULTS = None

_prog_cache = {}


def _build_program():
    import concourse.tile as tile
    from concourse import bacc, mybir

    nc = bacc.Bacc("TRN2", target_bir_lowering=False, debug=False,
                   num_devices=NCORES)
    f32 = mybir.dt.float32
    f32r = mybir.dt.float32r

    xp_d = nc.dram_tensor("xp", [SPC, CT, P, HP, WP], f32r,
                          kind="ExternalInput").ap()
    wt_d = nc.dram_tensor("wt", [NSLOT, CT, P, 9, CT, P], f32r,
                          kind="ExternalInput").ap()
    bs_d = nc.dram_tensor("bs", [P, SPC * CT], f32,
                          kind="ExternalInput").ap()
    out_d = nc.dram_tensor("out", [SPC, CT, P, H * W_], f32,
                           kind="ExternalOutput").ap()

    with tile.TileContext(nc) as tc:
        with tc.tile_pool(name="xpool", bufs=3) as xpool, \
             tc.tile_pool(name="xfpool", bufs=9) as xfpool, \
             tc.tile_pool(name="wpool", bufs=4) as wpool, \
             tc.tile_pool(name="bpool", bufs=1) as bpool, \
             tc.tile_pool(name="opool", bufs=8) as opool, \
             tc.tile_pool(name="psum", bufs=8, space="PSUM") as psum_pool:

            # PE warmup: dummy fp32 matmuls on a memset tile keep the PE
            # busy during the initial DMA fill so the HAM clock gate opens
            # (1.2 -> 2.4 GHz) before the first real matmul.
            scratch = bpool.tile([P, P], f32, name="scratch")
            nc.gpsimd.memset(scratch[:], 0.0)
            ps_warm = psum_pool.tile([P, NHALF], f32, name="ps_warm", tag="ps")
            for _ in range(WARMUP):
                nc.tensor.matmul(ps_warm[:, :P], scratch[:], scratch[:],
                                 start=True, stop=True, skip_group_check=True)

            bt = bpool.tile([P, SPC * CT], f32)


            def tap_aps(xts, ci_t, tap, ch, pstile, xfs=None):
                # Output row 0 (chunk 0) reads only the zero pad row for
                # dy=0 taps, row 31 (chunk 1) only for dy=2: shrink those
                # matmuls to 15 rows (N=480). start=True clears has_written
                # for the whole bank, and the full-width dy=1 taps overwrite
                # the untouched columns, so partial-range accumulation is
                # sound.
                dy, dx = divmod(tap, 3)
                r0, r1 = 16 * ch, 16 * ch + 16
                c0, c1 = 0, NHALF
                if ch == 0 and dy == 0:
                    r0, c0 = r0 + 1, 32
                elif ch == 1 and dy == 2:
                    r1, c1 = r1 - 1, NHALF - 32
                if xfs is not None:
                    # all taps read a shifted flat copy: a 1-D rhs AP is
                    # ~9ns/MM cheaper than the strided window
                    f0 = (dy - 1 + r0) * W_
                    rhs = xfs[ci_t][dx][:, f0: f0 + (r1 - r0) * W_]
                else:
                    rhs = xts[ci_t][:, dy + r0: dy + r1, dx: dx + W_]
                return rhs, pstile[:, c0:c1]

            wslots = {}

            def load_wslot(slot, split):
                tiles = []
                for ci_t in range(CT):
                    wtile = wpool.tile([P, 9, CT, P], f32r,
                                       name=f"wt{slot}_{ci_t}", tag="wt")
                    if split:
                        # three pieces: the first matmuls only need low taps
                        for g in range(3):
                            nc.scalar.dma_start(
                                wtile[:, 3 * g: 3 * g + 3],
                                wt_d[slot, ci_t][:, 3 * g: 3 * g + 3])
                    else:
                        nc.scalar.dma_start(wtile[:], wt_d[slot, ci_t])
                    tiles.append(wtile)
                wslots[slot] = tiles
                return tiles

            for s in range(SPC):
                xts = []
                for ci_t in range(CT):
                    xt = xpool.tile([P, HP, WP], f32r, name=f"xt{s}_{ci_t}",
                                    tag="xt")
                    if s == 0:
                        # row-split so the chunk-0 matmuls start sooner
                        nc.sync.dma_start(xt[:, :18], xp_d[s, ci_t][:, :18])
                        nc.sync.dma_start(xt[:, 18:], xp_d[s, ci_t][:, 18:])
                    else:
                        nc.sync.dma_start(xt[:], xp_d[s, ci_t])
                    xts.append(xt)
                xfs = None
                if s > 0:
                    # three column-shifted flat copies of the interior; the
                    # padding columns encode the dx shifts with zeros
                    xfs = []
                    for ci_t in range(CT):
                        trio = []
                        for dx in range(3):
                            xf = xfpool.tile([P, H * W_], f32r,
                                             name=f"xf{s}_{ci_t}_{dx}",
                                             tag="xf")
                            nc.vector.tensor_copy(
                                xf[:], xts[ci_t][:, 1:1 + H, dx: dx + W_])
                            trio.append(xf)
                        xfs.append(trio)
                slot = SLOT_OF[s]
                wts = wslots.get(slot) or load_wslot(slot, split=(s == 0))
                if s == 0:
                    nc.scalar.dma_start(bt[:], bs_d[:])

                ps = [[psum_pool.tile([P, NHALF], f32,
                                      name=f"ps{s}_{co_t}_{ch}", tag="ps")
                       for ch in range(2)] for co_t in range(CT)]

                last = s == SPC - 1
                if s == 0:
                    # (ch,ci) staged: first 18 matmuls only need x rows 0-17
                    # + ci0 weights (earliest start); ci1 not needed until
                    # half the sample is done (long prefetch window)
                    for ch, ci_t in ((0, 0), (1, 0), (0, 1), (1, 1)):
                        for tap in range(9):
                            for co_t in range(CT):
                                rhs, out_ap = tap_aps(xts, ci_t, tap, ch,
                                                      ps[co_t][ch])
                                nc.tensor.matmul(
                                    rhs=rhs, out=out_ap,
                                    lhsT=wts[ci_t][:, tap, co_t, :],
                                    start=(ci_t == 0 and tap == 0),
                                    stop=(ci_t == CT - 1 and tap == 8))
                    groups = [(co_t, ch) for co_t in range(CT)
                              for ch in range(2)]
                    mm_done = True
                elif not last:
                    # ci-outer: only ci_t=0 tiles gate the first 36 matmuls
                    for ci_t in range(CT):
                        for tap in range(9):
                            for co_t in range(CT):
                                lhsT = wts[ci_t][:, tap, co_t, :]
                                for ch in range(2):
                                    rhs, out_ap = tap_aps(xts, ci_t, tap, ch,
                                                          ps[co_t][ch], xfs)
                                    nc.tensor.matmul(
                                        rhs=rhs, out=out_ap, lhsT=lhsT,
                                        start=(ci_t == 0 and tap == 0),
                                        stop=(ci_t == CT - 1 and tap == 8))
                    groups = [(co_t, ch) for co_t in range(CT)
                              for ch in range(2)]
                    mm_done = True
                else:
                    groups = [(co_t, ch) for co_t in range(CT)
                              for ch in range(2)]
                    mm_done = False

                for co_t, ch in groups:
                    if not mm_done:
                        # last sample: emit each psum group's matmuls just
                        # before its drain so only one group lands in the tail
                        for ci_t in range(CT):
                            for tap in range(9):
                                rhs, out_ap = tap_aps(xts, ci_t, tap, ch,
                                                      ps[co_t][ch], xfs)
                                nc.tensor.matmul(
                                    rhs=rhs, out=out_ap,
                                    lhsT=wts[ci_t][:, tap, co_t, :],
                                    start=(ci_t == 0 and tap == 0),
                                    stop=(ci_t == CT - 1 and tap == 8))
                    ot = opool.tile([P, NHALF], f32,
                                    name=f"ot{s}_{co_t}_{ch}", tag="ot")
                    nc.vector.tensor_scalar_add(
                        ot[:], ps[co_t][ch][:],
                        bt[:, CT * s + co_t: CT * s + co_t + 1])
                    nc.sync.dma_start(
                        out_d[s, co_t][:, NHALF * ch: NHALF * (ch + 1)],
                        ot[:])
    nc.compile()
    return nc


def _plan_routing(arc):
    """Group the 64 samples into 24 same-branch pairs + 16 singles and lay
    them out per core as [p0,p0,p1,p1,p2,p2,s0,s1]. Returns (perm, wslot_src)
    where perm[core*8+pos] = original sample index and wslot_src[core*5+k] =
    original sample whose branch fills weight slot k of that core."""
    groups = [list(np.nonzero(arc == b)[0]) for b in range(4)]
    pairs, singles = [], []
    for g in groups:
        n2 = (len(g) // 2) * 2
        pairs.extend((g[i], g[i + 1]) for i in range(0, n2, 2))
        singles.extend(g[n2:])
    # move surplus pairs beyond 24 back to singles (keep exactly 24 pairs)
    while len(pairs) > 3 * NCORES:
        a, bb = pairs.pop()
        singles.extend([a, bb])
    assert len(pairs) == 3 * NCORES and len(singles) == 2 * NCORES
    perm = np.empty(B, np.int64)
    wsrc = np.empty(NCORES * NSLOT, np.int64)
    for c in range(NCORES):
        ps_ = pairs[3 * c: 3 * c + 3]
        sg = singles[2 * c: 2 * c + 2]
        samp = [ps_[0][0], ps_[0][1], ps_[1][0], ps_[1][1],
                ps_[2][0], ps_[2][1], sg[0], sg[1]]
        perm[8 * c: 8 * c + 8] = samp
        wsrc[NSLOT * c: NSLOT * (c + 1)] = [ps_[0][0], ps_[1][0], ps_[2][0],
                                            sg[0], sg[1]]
    return perm, wsrc


def kernel(x, sample_arc, W, b):
    global LAST_RESULTS
    from concourse.bass_utils import run_bass_kernel_spmd

    x = np.asarray(x, dtype=np.float32)
    arc = np.asarray(sample_arc)
    W = np.asarray(W, dtype=np.float32)
    b = np.asarray(b, dtype=np.float32)

    nc = _prog_cache.get("nc")
    if nc is None:
        nc = _prog_cache["nc"] = _build_program()

    perm, wsrc = _plan_routing(arc)

    # packed x (padded), in permuted order
    xp = np.zeros((B, CT, P, HP, WP), np.float32)
    xp[:, :, :, 1:1 + H, 1:1 + W_] = x[perm].reshape(B, CT, P, H, W_)

    # per-slot weights: [ci, ky, kx, co] layout
    wsel = W[arc[wsrc]]                      # [40, co, ci, 3, 3]
    wt = np.ascontiguousarray(wsel.transpose(0, 2, 3, 4, 1))
    wt = wt.reshape(NCORES, NSLOT, CT, P, 9, CT, P)

    bsel = b[arc[perm]].reshape(NCORES, SPC, CT, P)

    in_maps = []
    for c in range(NCORES):
        in_maps.append({
            "xp": xp[c * SPC:(c + 1) * SPC],
            "wt": wt[c],
            "bs": np.ascontiguousarray(
                bsel[c].transpose(2, 0, 1).reshape(P, SPC * CT)),
        })

    res = run_bass_kernel_spmd(nc, in_maps, core_ids=list(range(NCORES)),
                               trace=TRACE, tmpdir=TRACE_DIR)
    LAST_RESULTS = res

    out_perm = np.concatenate(
        [res.results[c]["out"].reshape(SPC, C, H, W_) for c in range(NCORES)],
        axis=0)
    out = np.empty_like(out_perm)
    out[perm] = out_perm
    return out


# revision 31
# speedup vs baseline: 1.0476x; 1.0023x over previous
"""MoE-routed 3x3 conv (MixedLayerWithArc) on 8 TRN2 NeuronCores.

Reference semantics: out[i] = conv3x3(x[i], W[sample_arc[i]], b[sample_arc[i]])
(the dense all-branch + one-hot-mask reference computes exactly this).

Strategy:
  * Routing resolved on the HOST (sample_arc is host data): gather the
    selected branch's weights/bias per sample -> 1 conv per sample instead
    of 4 (4x less compute than the reference).
  * Data-parallel over batch: 8 samples per core x 8 cores.
  * Host groups same-branch samples into pairs: each core gets 3 pairs + 2
    singles -> 5 weight slots instead of 8 (weight DMA 18.9 -> 11.8 MB/core).
    Among 64 samples in 4 branches there are always >= 30 same-branch pairs,
    so 24 pairs for 8 cores always exist.
  * Conv as 9 shifted matmuls accumulated in PSUM, contracting over C_in
    (256 = 2 partition tiles of 128). dtype float32r: 4x PE throughput vs
    float32 at ~1e-4 relative error.
  * Host pre-pads x to 34x34 so DMAs are contiguous and each tap is a
    strided SBUF view; bias folds into the PSUM->SBUF eviction.
  * PE warmup matmuls run during the initial DMA fill (HAM clock ramp).

Per-core inputs:
  xp  [8, 2, 128, 34, 34] f32   padded input   (sample, ci_tile, ci, h, w)
  wt  [5, 2, 128, 9, 2, 128] f32 weights       (slot, ci_tile, ci, tap, co_tile, co)
  bs  [128, 16] f32              bias          (co, sample*2 + co_tile)
  out [8, 2, 128, 1024] f32                    (sample, co_tile, co, h*w)
"""
import numpy as np

B, C, H, W_ = 64, 256, 32, 32
NCORES = 8
SPC = B // NCORES          # samples per core
HP, WP = H + 2, W_ + 2     # padded spatial
P = 128                    # partition tile
CT = C // P                # channel tiles (2)
NHALF = H * W_ // 2        # 512 = one PSUM bank of fp32
NSLOT = 5                  # weight slots per core (3 pairs + 2 singles)
SLOT_OF = [0, 0, 1, 1, 2, 2, 3, 4]   # sample -> weight slot (static)
WARMUP = 16

TRACE = False
TRACE_DIR = None
LAST_RESULTS = None

_prog_cache = {}


def _build_program():
    import concourse.tile as tile
    from concourse import bacc, mybir

    nc = bacc.Bacc("TRN2", target_bir_lowering=False, debug=False,
                   num_devices=NCORES)
    f32 = mybir.dt.float32
    f32r = mybir.dt.float32r

    xp_d = nc.dram_tensor("xp", [SPC, CT, P, HP, WP], f32r,
                          kind="ExternalInput").ap()
    wt_d = nc.dram_tensor("wt", [NSLOT, CT, P, 9, CT, P], f32r,
                          kind="ExternalInput").ap()
    bs_d = nc.dram_tensor("bs", [P, SPC * CT], f32,
                          kind="ExternalInput").ap()
    out_d = nc.dram_tensor("out", [SPC, CT, P, H * W_], f32,
                           kind="ExternalOutput").ap()

    with tile.TileContext(nc) as tc:
        with tc.tile_pool(name="xpool", bufs=3) as xpool, \
             tc.tile_pool(name="xfpool", bufs=9) as xfpool, \
             tc.tile_pool(name="wpool", bufs=4) as wpool, \
             tc.tile_pool(name="bpool", bufs=1) as bpool, \
             tc.tile_pool(name="opool", bufs=8) as opool, \
             tc.tile_pool(name="psum", bufs=8, space="PSUM") as psum_pool:

            # PE warmup: dummy fp32 matmuls on a memset tile keep the PE
            # busy during the initial DMA fill so the HAM clock gate opens
            # (1.2 -> 2.4 GHz) before the first real matmul.
            scratch = bpool.tile([P, P], f32, name="scratch")
            nc.gpsimd.memset(scratch[:], 0.0)
            ps_warm = psum_pool.tile([P, NHALF], f32, name="ps_warm", tag="ps")
            for _ in range(WARMUP):
                nc.tensor.matmul(ps_warm[:, :P], scratch[:], scratch[:],
                                 start=True, stop=True, skip_group_check=True)

            bt = bpool.tile([P, SPC * CT], f32)


            def tap_aps(xts, ci_t, tap, ch, pstile, xfs=None):
                # Output row 0 (chunk 0) reads only the zero pad row for
                # dy=0 taps, row 31 (chunk 1) only for dy=2: shrink those
                # matmuls to 15 rows (N=480). start=True clears has_written
                # for the whole bank, and the full-width dy=1 taps overwrite
                # the untouched columns, so partial-range accumulation is
                # sound.
                dy, dx = divmod(tap, 3)
                r0, r1 = 16 * ch, 16 * ch + 16
                c0, c1 = 0, NHALF
                if ch == 0 and dy == 0:
                    r0, c0 = r0 + 1, 32
                elif ch == 1 and dy == 2:
                    r1, c1 = r1 - 1, NHALF - 32
                if xfs is not None:
                    # all taps read a shifted flat copy: a 1-D rhs AP is
                    # ~9ns/MM cheaper than the strided window
                    f0 = (dy - 1 + r0) * W_
                    rhs = xfs[ci_t][dx][:, f0: f0 + (r1 - r0) * W_]
                else:
                    rhs = xts[ci_t][:, dy + r0: dy + r1, dx: dx + W_]
                return rhs, pstile[:, c0:c1]

            wslots = {}

            def load_wslot(slot, split):
                tiles = []
                for ci_t in range(CT):
                    wtile = wpool.tile([P, 9, CT, P], f32r,
                                       name=f"wt{slot}_{ci_t}", tag="wt")
                    if split:
                        # three pieces: the first matmuls only need low taps
                        for g in range(3):
                            nc.scalar.dma_start(
                                wtile[:, 3 * g: 3 * g + 3],
                                wt_d[slot, ci_t][:, 3 * g: 3 * g + 3])
                    else:
                        nc.scalar.dma_start(wtile[:], wt_d[slot, ci_t])
                    tiles.append(wtile)
                wslots[slot] = tiles
                return tiles

            for s in range(SPC):
                xts = []
                for ci_t in range(CT):
                    xt = xpool.tile([P, HP, WP], f32r, name=f"xt{s}_{ci_t}",
                                    tag="xt")
                    if s == 0:
                        # row-split so the chunk-0 matmuls start sooner
                        nc.sync.dma_start(xt[:, :18], xp_d[s, ci_t][:, :18])
                        nc.sync.dma_start(xt[:, 18:], xp_d[s, ci_t][:, 18:])
                    else:
                        nc.sync.dma_start(xt[:], xp_d[s, ci_t])
                    xts.append(xt)
                # three column-shifted flat copies of the interior; the
                # padding columns encode the dx shifts with zeros
                if True:
                    xfs = []
                    for ci_t in range(CT):
                        trio = []
                        for dx in range(3):
                            xf = xfpool.tile([P, H * W_], f32r,
                                             name=f"xf{s}_{ci_t}_{dx}",
                                             tag="xf")
                            nc.vector.tensor_copy(
                                xf[:], xts[ci_t][:, 1:1 + H, dx: dx + W_])
                            trio.append(xf)
                        xfs.append(trio)
                slot = SLOT_OF[s]
                wts = wslots.get(slot) or load_wslot(slot, split=(s == 0))
                if s == 0:
                    nc.scalar.dma_start(bt[:], bs_d[:])

                ps = [[psum_pool.tile([P, NHALF], f32,
                                      name=f"ps{s}_{co_t}_{ch}", tag="ps")
                       for ch in range(2)] for co_t in range(CT)]

                last = s == SPC - 1
                if s == 0:
                    # (ch,ci) staged: first 18 matmuls only need x rows 0-17
                    # + ci0 weights (earliest start); ci1 not needed until
                    # half the sample is done (long prefetch window)
                    for ch, ci_t in ((0, 0), (1, 0), (0, 1), (1, 1)):
                        for tap in range(9):
                            for co_t in range(CT):
                                rhs, out_ap = tap_aps(
                                    xts, ci_t, tap, ch, ps[co_t][ch],
                                    None if (ch, ci_t) == (0, 0) else xfs)
                                nc.tensor.matmul(
                                    rhs=rhs, out=out_ap,
                                    lhsT=wts[ci_t][:, tap, co_t, :],
                                    start=(ci_t == 0 and tap == 0),
                                    stop=(ci_t == CT - 1 and tap == 8))
                    groups = [(co_t, ch) for co_t in range(CT)
                              for ch in range(2)]
                    mm_done = True
                elif not last:
                    # ci-outer: only ci_t=0 tiles gate the first 36 matmuls
                    for ci_t in range(CT):
                        for tap in range(9):
                            for co_t in range(CT):
                                lhsT = wts[ci_t][:, tap, co_t, :]
                                for ch in range(2):
                                    rhs, out_ap = tap_aps(xts, ci_t, tap, ch,
                                                          ps[co_t][ch], xfs)
                                    nc.tensor.matmul(
                                        rhs=rhs, out=out_ap, lhsT=lhsT,
                                        start=(ci_t == 0 and tap == 0),
                                        stop=(ci_t == CT - 1 and tap == 8))
                    groups = [(co_t, ch) for co_t in range(CT)
                              for ch in range(2)]
                    mm_done = True
                else:
                    groups = [(co_t, ch) for co_t in range(CT)
                              for ch in range(2)]
                    mm_done = False

                for co_t, ch in groups:
                    if not mm_done:
                        # last sample: emit each psum group's matmuls just
                        # before its drain so only one group lands in the tail
                        for ci_t in range(CT):
                            for tap in range(9):
                                rhs, out_ap = tap_aps(xts, ci_t, tap, ch,
                                                      ps[co_t][ch], xfs)
                                nc.tensor.matmul(
                                    rhs=rhs, out=out_ap,
                                    lhsT=wts[ci_t][:, tap, co_t, :],
                                    start=(ci_t == 0 and tap == 0),
                                    stop=(ci_t == CT - 1 and tap == 8))
                    ot = opool.tile([P, NHALF], f32,
                                    name=f"ot{s}_{co_t}_{ch}", tag="ot")
                    nc.vector.tensor_scalar_add(
                        ot[:], ps[co_t][ch][:],
                        bt[:, CT * s + co_t: CT * s + co_t + 1])
                    nc.sync.dma_start(
                        out_d[s, co_t][:, NHALF * ch: NHALF * (ch + 1)],
                        ot[:])
    nc.compile()
    return nc


def _plan_routing(arc):
    """Group the 64 samples into 24 same-branch pairs + 16 singles and lay
    them out per core as [p0,p0,p1,p1,p2,p2,s0,s1]. Returns (perm, wslot_src)
    where perm[core*8+pos] = original sample index and wslot_src[core*5+k] =
    original sample whose branch fills weight slot k of that core."""
    groups = [list(np.nonzero(arc == b)[0]) for b in range(4)]
    pairs, singles = [], []
    for g in groups:
        n2 = (len(g) // 2) * 2
        pairs.extend((g[i], g[i + 1]) for i in range(0, n2, 2))
        singles.extend(g[n2:])
    # move surplus pairs beyond 24 back to singles (keep exactly 24 pairs)
    while len(pairs) > 3 * NCORES:
        a, bb = pairs.pop()
        singles.extend([a, bb])
    assert len(pairs) == 3 * NCORES and len(singles) == 2 * NCORES
    perm = np.empty(B, np.int64)
    wsrc = np.empty(NCORES * NSLOT, np.int64)
    for c in range(NCORES):
        ps_ = pairs[3 * c: 3 * c + 3]
        sg = singles[2 * c: 2 * c + 2]
        samp = [ps_[0][0], ps_[0][1], ps_[1][0], ps_[1][1],
                ps_[2][0], ps_[2][1], sg[0], sg[1]]
        perm[8 * c: 8 * c + 8] = samp
        wsrc[NSLOT * c: NSLOT * (c + 1)] = [ps_[0][0], ps_[1][0], ps_[2][0],
                                            sg[0], sg[1]]
    return perm, wsrc


def kernel(x, sample_arc, W, b):
    global LAST_RESULTS
    from concourse.bass_utils import run_bass_kernel_spmd

    x = np.asarray(x, dtype=np.float32)
    arc = np.asarray(sample_arc)
    W = np.asarray(W, dtype=np.float32)
    b = np.asarray(b, dtype=np.float32)

    nc = _prog_cache.get("nc")
    if nc is None:
        nc = _prog_cache["nc"] = _build_program()

    perm, wsrc = _plan_routing(arc)

    # packed x (padded), in permuted order
    xp = np.zeros((B, CT, P, HP, WP), np.float32)
    xp[:, :, :, 1:1 + H, 1:1 + W_] = x[perm].reshape(B, CT, P, H, W_)

    # per-slot weights: [ci, ky, kx, co] layout
    wsel = W[arc[wsrc]]                      # [40, co, ci, 3, 3]
    wt = np.ascontiguousarray(wsel.transpose(0, 2, 3, 4, 1))
    wt = wt.reshape(NCORES, NSLOT, CT, P, 9, CT, P)

    bsel = b[arc[perm]].reshape(NCORES, SPC, CT, P)

    in_maps = []
    for c in range(NCORES):
        in_maps.append({
            "xp": xp[c * SPC:(c + 1) * SPC],
            "wt": wt[c],
            "bs": np.ascontiguousarray(
                bsel[c].transpose(2, 0, 1).reshape(P, SPC * CT)),
        })

    res = run_bass_kernel_spmd(nc, in_maps, core_ids=list(range(NCORES)),
                               trace=TRACE, tmpdir=TRACE_DIR)
    LAST_RESULTS = res

    out_perm = np.concatenate(
        [res.results[c]["out"].reshape(SPC, C, H, W_) for c in range(NCORES)],
        axis=0)
    out = np.empty_like(out_perm)
    out[perm] = out_perm
    return out
